# revision 21
# baseline (speedup 1.0000x reference)
"""CascadeHadamardLinear Trainium2 kernel (8-core data-parallel over tokens).

Math per token row x[4096]:
  x_rot = (x * S_in) @ blockdiag(H_128)     H = sign/sqrt(128) Hadamard
  x_q   = NVFP4 fake-quant of x_rot (16-elem blocks, e2m1 snap, RNE)
  out   = x_q @ W^T + (x_rot @ la^T) @ lb^T + bias

Device computes with x_rot~ = sqrt(128)*x_rot via an EXACT +/-1 sign
matrix (S folded in) in bf16, with x split host-side into x_hi + x_lo
(dual bf16) for fp32-class rotation at bf16 matmul rate. The quantizer
is scale-invariant (z = 6*v/amax), so f is unchanged; sqrt(128) is
folded out of the weights on the host (w~ = w/sqrt(128)).

Per core (1024 tokens, halves of 256/768 for phase overlap):
  p1(h): per jg (4 hadamard blocks): DMA x_hi/x_lo; LoRA1 t1 += la_eff^T
    @ x_hi (la_eff host-precomputed, includes S, H, 1/sqrt(128)); per
    128-token tile: 8 bf16 rotation MMs -> PSUM bank [128,512]; quant:
    absmax-16 (DVE), reciprocal (DVE), rs6/sc (ACT muls), fused
    z*rs6 + 3-way e2m1 magic-add snap in ONE custom DVE op, xq = f*sc
    -> bf16 (GPSIMD); PE-transpose xq -> xqT[d-major].
  p2(h): per og (512 out cols): stream w~ (bf16, host layout
    [c,og,j,o] = 32KB contiguous lines); per 128-token tile: PSUM
    accumulation of 32 main MMs + 1 merged (LoRA2+bias) K=33 MM
    (t1T has a ones row, lbT an appended bias row); ACT evac, DMA out.
Emission p1(0), w-prefetch, p2(0), p1(1), p2(1): Tile backfills PE
idle in DVE-bound p1 windows with ready p2 matmuls.
"""

import os
import sys

for _p in ("/opt/trn_rl_repo",):
    if os.path.isdir(_p) and _p not in sys.path:
        sys.path.insert(0, _p)

import numpy as np

import concourse.bass as bass
import concourse.mybir as mybir
import concourse.tile as tile
from concourse import bacc
from concourse.bass_utils import run_bass_kernel_spmd

F32 = np.float32

# ---------------- problem constants (hardcoded per contract) ----------------
B, S, D_IN, D_OUT, RANK, HBS = 4, 2048, 4096, 4096, 32, 128
NTOK = B * S                  # 8192
NCORES = 8
NT = NTOK // NCORES           # 1024 tokens per core
NJ = D_IN // HBS              # 32 hadamard blocks
QB = 16                       # quant block size
OG_N = D_OUT // 512           # 8 output column groups
H0 = 256                      # prologue half (tokens)

# quant snap constants (1.5*2^k magic so ulp is uniform on both sides of c)
C_HALF = 6291456.0            # 1.5*2^22, ulp 0.5
C_INT = 12582912.0            # 1.5*2^23, ulp 1.0
C_EVEN = 25165824.0           # 1.5*2^24, ulp 2.0
THF = 5.0625                  # 2.25^2
TH23 = 20.25                  # 4.5^2


# ---------------- custom DVE ops (e2m1 level snap, 2 passes, no t2) --------
def _register_snap_ops():
    from concourse.dve_spec import (
        Spec, Src0, Src1, C0, C1, C2, lower as dve_lower, sq, select, _has_src1,
    )
    from concourse.dve_ops import (
        DveOp, OPS, CUSTOM_DVE_SPECS, _SUB_OPCODE_FOR_NAME, _CUSTOM_DVE_ROW_BASE,
    )
    from concourse.dve_uop import DveOpSpec
    from concourse.dve_table_gen import dve_ver_for

    def _ref_a(in0, in1, c0, c1, c2):
        z = in0.astype(F32)
        r_int = (z + F32(c1)) - F32(c1)
        r_even = (z + F32(c2)) - F32(c2)
        return np.where(z * z < F32(c0), r_int, r_even).astype(F32)

    def _ref_b(in0, in1, c0, c1, c2):
        z = in0.astype(F32)
        r_half = (z + F32(c1)) - F32(c1)
        return np.where(z * z < F32(c0), r_half, in1.astype(F32)).astype(F32)

    def _mk(name, body, ref):
        if name in _SUB_OPCODE_FOR_NAME:
            return next(op for op in OPS if op.name == name)
        spec = Spec(body=body, reference=ref)
        row = _CUSTOM_DVE_ROW_BASE + len(OPS)
        assert row < 0x20
        ver = dve_ver_for("TRN2")
        uops = dve_lower(spec, ver=ver)
        sha = DveOpSpec(
            name=name, opcode=row, uops=uops, rd1_en=_has_src1(spec)
        ).sha(ver)
        op = DveOp(name, spec, subdim=False, uops_sha={ver: sha})
        OPS.append(op)
        CUSTOM_DVE_SPECS[name] = spec
        _SUB_OPCODE_FOR_NAME[name] = row
        return op

    z = Src0
    snap_a = _mk(
        "SNAP_A_ANT",
        select(sq(z) < C0, (z + C1) - C1, (z + C2) - C2),
        _ref_a,
    )
    snap_b = _mk(
        "SNAP_B_ANT",
        select(sq(z) < C0, (z + C1) - C1, Src1),
        _ref_b,
    )
    return snap_a, snap_b


SNAP_A, SNAP_B = _register_snap_ops()


# ---------------- device kernel ----------------
def _build_nc():
    nc = bacc.Bacc(
        "TRN2", target_bir_lowering=False, debug=False, num_devices=NCORES
    )
    dt = mybir.dt
    # x packed per (jg, chunk): [c, jg, seg] with seg = (dj, t) contiguous per
    # chunk (c0=128, c1=128, c2=768 tokens) so every DMA line is >=1KB
    xhi = nc.dram_tensor("xhi", [HBS, NJ // 4, 4 * NT], dt.bfloat16, kind="ExternalInput")
    xlo = nc.dram_tensor("xlo", [HBS, NJ // 4, 4 * NT], dt.bfloat16, kind="ExternalInput")
    Hs = nc.dram_tensor("Hs", [HBS, NJ, HBS], dt.bfloat16, kind="ExternalInput")
    laE = nc.dram_tensor("laE", [HBS, NJ, RANK], dt.bfloat16, kind="ExternalInput")
    w4 = nc.dram_tensor("w4", [HBS, OG_N, NJ, 512], dt.bfloat16, kind="ExternalInput")
    lbA = nc.dram_tensor("lbA", [RANK + 1, OG_N, 512], dt.bfloat16, kind="ExternalInput")
    y = nc.dram_tensor("y", [NT, D_OUT], dt.float32, kind="ExternalOutput")

    with tile.TileContext(nc) as tc:
        _emit(nc, tc, xhi, xlo, Hs, laE, w4, lbA, y)
    nc.compile()
    return nc


def _emit(nc, tc, xhi, xlo, Hs, laE, w4, lbA, y):
    from contextlib import ExitStack

    dt = mybir.dt
    Alu = mybir.AluOpType
    Act = mybir.ActivationFunctionType

    with ExitStack() as ctx:
        consts = ctx.enter_context(tc.tile_pool(name="consts", bufs=1))

        Hs_sb = consts.tile([HBS, NJ, HBS], dt.bfloat16)
        laE_sb = consts.tile([HBS, NJ, RANK], dt.bfloat16)
        nc.sync.dma_start(out=laE_sb[:], in_=laE[:])
        lbA_sb = consts.tile([RANK + 1, OG_N, 512], dt.bfloat16)
        nc.scalar.dma_start(out=lbA_sb[:], in_=lbA[:])
        ident_bf = consts.tile([128, 128], dt.bfloat16)
        from concourse.masks import make_identity
        make_identity(nc, ident_bf[:])

        t1T = consts.tile([RANK + 1, NT], dt.bfloat16)
        nc.vector.memset(t1T[RANK : RANK + 1, :], 1.0)

        # xqT[c, j, t] : feature-major quantized activations (bf16)
        xqT = consts.tile([HBS, NJ, NT], dt.bfloat16)

        # working pools
        xh_pool = ctx.enter_context(tc.tile_pool(name="xh", bufs=2))
        xl_pool = ctx.enter_context(tc.tile_pool(name="xl", bufs=2))
        qsm = ctx.enter_context(tc.tile_pool(name="qsm", bufs=4))
        z_pool = ctx.enter_context(tc.tile_pool(name="z", bufs=3))
        r23_pool = ctx.enter_context(tc.tile_pool(name="r23", bufs=2))
        f_pool = ctx.enter_context(tc.tile_pool(name="f", bufs=3))
        xq_pool = ctx.enter_context(tc.tile_pool(name="xq", bufs=8))
        wbf_pool = ctx.enter_context(tc.tile_pool(name="wbf", bufs=2))
        out_pool = ctx.enter_context(tc.tile_pool(name="out", bufs=3))
        rot_ps = ctx.enter_context(tc.tile_pool(name="rotps", bufs=2, space="PSUM"))
        tr_ps = ctx.enter_context(tc.tile_pool(name="trps", bufs=2, space="PSUM"))
        t1_ps = ctx.enter_context(tc.tile_pool(name="t1ps", bufs=2, space="PSUM"))
        out_ps = ctx.enter_context(tc.tile_pool(name="outps", bufs=2, space="PSUM"))

        def emit_w_loads(h):
            tiles = []
            for og in range(OG_N):
                wt = wbf_pool.tile([HBS, NJ, 512], dt.bfloat16,
                                   name=f"w{h}_{og}", tag="wbf")
                nc.scalar.dma_start(out=wt[:], in_=w4[:, og])
                tiles.append(wt)
            return tiles

        def emit_p1(h, t0, tlen, seg0, load_hs=False):
            hsl = slice(t0, t0 + tlen)
            ts_n = tlen // 128
            # LoRA1 accumulators in <=512-token chunks (one PSUM bank each)
            lchunks = []
            o = 0
            while o < tlen:
                w = min(512, tlen - o)
                acc = t1_ps.tile([RANK, w], dt.float32,
                                 name=f"t1acc{h}_{o}", tag="t1acc")
                lchunks.append((o, w, acc))
                o += w
            for jg in range(NJ // 4):
                seg = slice(seg0, seg0 + 4 * tlen)
                xh = xh_pool.tile([HBS, 4, tlen], dt.bfloat16,
                                  name=f"xh{h}_{jg}", tag="xh")
                nc.sync.dma_start(
                    out=xh[:],
                    in_=xhi[:, jg, seg].rearrange("c (d t) -> c d t", d=4),
                )
                xl = xl_pool.tile([HBS, 4, tlen], dt.bfloat16,
                                  name=f"xl{h}_{jg}", tag="xl")
                nc.sync.dma_start(
                    out=xl[:],
                    in_=xlo[:, jg, seg].rearrange("c (d t) -> c d t", d=4),
                )
                if load_hs:
                    nc.sync.dma_start(
                        out=Hs_sb[:, 4 * jg : 4 * jg + 4, :],
                        in_=Hs[:, 4 * jg : 4 * jg + 4, :],
                    )
                for dj in range(4):
                    j = 4 * jg + dj
                    for (o, w, acc) in lchunks:
                        nc.tensor.matmul(
                            acc[:], lhsT=laE_sb[:, j, :],
                            rhs=xh[:, dj, o : o + w],
                            start=(j == 0), stop=(j == NJ - 1),
                        )
                xq_tiles = []
                for ts in range(ts_n):
                    tsl = slice(ts * 128, (ts + 1) * 128)
                    bank = rot_ps.tile([128, 512], dt.float32,
                                       name=f"bank{h}_{jg}_{ts}", tag="bank")
                    for dj in range(4):
                        j = 4 * jg + dj
                        nc.tensor.matmul(
                            bank[:, dj * HBS : (dj + 1) * HBS],
                            lhsT=xh[:, dj, tsl], rhs=Hs_sb[:, j, :],
                            start=(dj == 0), stop=False,
                        )
                        nc.tensor.matmul(
                            bank[:, dj * HBS : (dj + 1) * HBS],
                            lhsT=xl[:, dj, tsl], rhs=Hs_sb[:, j, :],
                            start=False, stop=(dj == 3),
                        )
                    nb = 512 // QB
                    amax = qsm.tile([128, nb], dt.float32, name=f"am{h}{jg}{ts}", tag="amax")
                    nc.vector.tensor_reduce(
                        out=amax[:], in_=bank[:].rearrange("p (b s) -> p b s", s=QB),
                        axis=mybir.AxisListType.X, op=Alu.max,
                        apply_absolute_value=True,
                    )
                    ra = qsm.tile([128, nb], dt.float32, name=f"ra{h}{jg}{ts}", tag="ra")
                    nc.vector.reciprocal(out=ra[:], in_=amax[:])
                    rs6 = qsm.tile([128, nb], dt.float32, name=f"rs6{h}{jg}{ts}", tag="rs6")
                    nc.scalar.mul(out=rs6[:], in_=ra[:], mul=6.0)
                    sc = qsm.tile([128, nb], dt.float32, name=f"sc{h}{jg}{ts}", tag="sc")
                    nc.scalar.mul(out=sc[:], in_=amax[:], mul=1.0 / 6.0)
                    z = z_pool.tile([128, 512], dt.float32, name=f"z{h}{jg}{ts}", tag="z")
                    nc.vector.tensor_tensor(
                        out=z[:].rearrange("p (b s) -> p b s", s=QB),
                        in0=bank[:].rearrange("p (b s) -> p b s", s=QB),
                        in1=rs6[:].unsqueeze(2).broadcast_to([128, nb, QB]),
                        op=Alu.mult,
                    )
                    r23 = r23_pool.tile([128, 512], dt.float32, name=f"r23{h}{jg}{ts}", tag="r23")
                    nc.vector._custom_dve(
                        SNAP_A, out=r23[:], in0=z[:], s0=TH23, s1=C_INT, imm2=C_EVEN,
                    )
                    f = f_pool.tile([128, 512], dt.float32, name=f"f{h}{jg}{ts}", tag="f")
                    nc.vector._custom_dve(
                        SNAP_B, out=f[:], in0=z[:], in1=r23[:], s0=THF, s1=C_HALF,
                    )
                    xq_t = xq_pool.tile([128, 512], dt.bfloat16, name=f"xq{h}{jg}{ts}", tag="xq")
                    nc.gpsimd.tensor_tensor(
                        out=xq_t[:].rearrange("p (b s) -> p b s", s=QB),
                        in0=f[:].rearrange("p (b s) -> p b s", s=QB),
                        in1=sc[:].unsqueeze(2).broadcast_to([128, nb, QB]),
                        op=Alu.mult,
                    )
                    xq_tiles.append(xq_t)
                for dj in range(4):
                    j = 4 * jg + dj
                    pt = tr_ps.tile([128, tlen], dt.bfloat16, name=f"pt{h}_{j}", tag="pt")
                    for ts in range(ts_n):
                        nc.tensor.matmul(
                            pt[:, ts * 128 : (ts + 1) * 128],
                            lhsT=xq_tiles[ts][:, dj * HBS : (dj + 1) * HBS],
                            rhs=ident_bf[:], is_transpose=True,
                            start=(ts == 0), stop=(ts == ts_n - 1),
                        )
                    nc.scalar.copy(out=xqT[:, j, hsl], in_=pt[:])
            for (o, w, acc) in lchunks:
                nc.scalar.copy(out=t1T[:RANK, t0 + o : t0 + o + w], in_=acc[:])

        def emit_p2(h, t0, tlen, wtiles):
            ts_n = tlen // 128
            for og in range(OG_N):
                wt = wtiles[og]
                for th in range(ts_n):
                    tsl = slice(t0 + th * 128, t0 + (th + 1) * 128)
                    po = out_ps.tile([128, 512], dt.float32,
                                     name=f"po{h}{og}{th}", tag="po")
                    for k in range(NJ):
                        nc.tensor.matmul(
                            po[:], lhsT=xqT[:, k, tsl], rhs=wt[:, k, :],
                            start=(k == 0), stop=False,
                        )
                    nc.tensor.matmul(
                        po[:], lhsT=t1T[:, tsl], rhs=lbA_sb[:, og, :],
                        start=False, stop=True,
                    )
                    ot = out_pool.tile([128, 512], dt.float32,
                                       name=f"ot{h}{og}{th}", tag="ot")
                    nc.scalar.copy(out=ot[:], in_=po[:])
                    nc.sync.dma_start(
                        out=y[tsl, og * 512 : (og + 1) * 512], in_=ot[:]
                    )

        w0 = emit_w_loads(0)
        emit_p1(0, 0, 128, 0, load_hs=True)     # chunk c0
        emit_p1(1, 128, 128, 512)               # chunk c1
        emit_p1(2, H0, NT - H0, 1024)           # chunk c2 (= second half)
        w1 = emit_w_loads(1)
        emit_p2(0, 0, H0, w0)
        emit_p2(1, H0, NT - H0, w1)


_NC_CACHE = None


def _get_nc():
    global _NC_CACHE
    if _NC_CACHE is None:
        _NC_CACHE = _build_nc()
    return _NC_CACHE


# ---------------- host wrapper ----------------
def _prep_inputs(x, S_in, H_block, w_quantized, lora_a, lora_b, bias):
    import ml_dtypes
    BF16 = ml_dtypes.bfloat16
    x = np.asarray(x, dtype=F32)
    S_in = np.asarray(S_in, dtype=F32)
    H_block = np.asarray(H_block, dtype=F32)
    w_quantized = np.asarray(w_quantized, dtype=F32)
    lora_a = np.asarray(lora_a, dtype=F32)
    lora_b = np.asarray(lora_b, dtype=F32)
    bias = np.asarray(bias, dtype=F32)

    x_flat = x.reshape(NTOK, D_IN)
    x_hi = x_flat.astype(BF16)
    x_lo = (x_flat - x_hi.astype(F32)).astype(BF16)

    Ssq = S_in.reshape(NJ, HBS).T                        # [c, j]
    Hsign = np.sign(H_block).astype(F32)                 # +/-1 exact
    Hs = (Ssq[:, :, None] * Hsign[:, None, :]).astype(BF16)  # [c, j, c']

    la3 = lora_a.reshape(RANK, NJ, HBS)                  # [r, j, c']
    la_eff = np.einsum(
        "cd,rjd->cjr", H_block.astype(np.float64), la3.astype(np.float64)
    )
    laE = (Ssq[:, :, None] * la_eff.astype(F32)).astype(BF16)  # [c, j, r]

    rinv = np.float64(1.0) / np.sqrt(np.float64(HBS))
    # w4[c, og, j, o] = w[og*512+o, j*128+c] / sqrt(128)
    w4 = np.ascontiguousarray(
        (w_quantized.astype(np.float64) * rinv)
        .astype(F32)
        .reshape(OG_N, 512, NJ, HBS)
        .transpose(3, 0, 2, 1)
        .astype(BF16)
    )
    lbA = np.concatenate(
        [lora_b.T, bias.reshape(1, D_OUT)], axis=0
    ).reshape(RANK + 1, OG_N, 512).astype(BF16)
    lbA = np.ascontiguousarray(lbA)

    # chunk segmentation: (seg_offset, t0, tlen) — must match device emission
    CHUNKS = [(0, 0, 128), (512, 128, 128), (1024, 256, NT - 256)]

    def _pack(xv):
        # [NTOK, D] -> per-core [c, jg, seg] with seg=(chunk|dj|t) contiguous
        x5 = xv.reshape(NT, NJ, HBS).transpose(2, 1, 0)  # [c, j, t]
        x6 = np.empty((HBS, NJ // 4, 4 * NT), dtype=xv.dtype)
        for jg in range(NJ // 4):
            blk = x5[:, 4 * jg : 4 * jg + 4, :]           # [c, 4, NT]
            for (s0, t0, tl) in CHUNKS:
                x6[:, jg, s0 : s0 + 4 * tl] = blk[:, :, t0 : t0 + tl].reshape(HBS, 4 * tl)
        return np.ascontiguousarray(x6)

    per_core = []
    for c in range(NCORES):
        tsl = slice(c * NT, (c + 1) * NT)
        per_core.append(
            {"xhi": _pack(x_hi[tsl]), "xlo": _pack(x_lo[tsl]),
             "Hs": Hs, "laE": laE, "w4": w4, "lbA": lbA}
        )
    return per_core


def kernel(x, S_in, H_block, w_quantized, lora_a, lora_b, bias):
    in_maps = _prep_inputs(x, S_in, H_block, w_quantized, lora_a, lora_b, bias)
    nc = _get_nc()
    res = run_bass_kernel_spmd(nc, in_maps, core_ids=list(range(NCORES)))
    out = np.concatenate([res.results[c]["y"] for c in range(NCORES)], axis=0)
    return out.reshape(B, S, D_OUT).astype(F32)


# revision 22
# speedup vs baseline: 1.0422x; 1.0422x over previous
"""CascadeHadamardLinear Trainium2 kernel (8-core data-parallel over tokens).

Math per token row x[4096]:
  x_rot = (x * S_in) @ blockdiag(H_128)     H = sign/sqrt(128) Hadamard
  x_q   = NVFP4 fake-quant of x_rot (16-elem blocks, e2m1 snap, RNE)
  out   = x_q @ W^T + (x_rot @ la^T) @ lb^T + bias

Device computes with x_rot~ = sqrt(128)*x_rot via an EXACT +/-1 sign
matrix (S folded in) in bf16, with x split host-side into x_hi + x_lo
(dual bf16) for fp32-class rotation at bf16 matmul rate. The quantizer
is scale-invariant (z = 6*v/amax), so f is unchanged; sqrt(128) is
folded out of the weights on the host (w~ = w/sqrt(128)).

Per core (1024 tokens, halves of 256/768 for phase overlap):
  p1(h): per jg (4 hadamard blocks): DMA x_hi/x_lo; LoRA1 t1 += la_eff^T
    @ x_hi (la_eff host-precomputed, includes S, H, 1/sqrt(128)); per
    128-token tile: 8 bf16 rotation MMs -> PSUM bank [128,512]; quant:
    absmax-16 (DVE), reciprocal (DVE), rs6/sc (ACT muls), fused
    z*rs6 + 3-way e2m1 magic-add snap in ONE custom DVE op, xq = f*sc
    -> bf16 (GPSIMD); PE-transpose xq -> xqT[d-major].
  p2(h): per og (512 out cols): stream w~ (bf16, host layout
    [c,og,j,o] = 32KB contiguous lines); per 128-token tile: PSUM
    accumulation of 32 main MMs + 1 merged (LoRA2+bias) K=33 MM
    (t1T has a ones row, lbT an appended bias row); ACT evac, DMA out.
Emission p1(0), w-prefetch, p2(0), p1(1), p2(1): Tile backfills PE
idle in DVE-bound p1 windows with ready p2 matmuls.
"""

import os
import sys

for _p in ("/opt/trn_rl_repo",):
    if os.path.isdir(_p) and _p not in sys.path:
        sys.path.insert(0, _p)

import numpy as np

import concourse.bass as bass
import concourse.mybir as mybir
import concourse.tile as tile
from concourse import bacc
from concourse.bass_utils import run_bass_kernel_spmd

F32 = np.float32

# ---------------- problem constants (hardcoded per contract) ----------------
B, S, D_IN, D_OUT, RANK, HBS = 4, 2048, 4096, 4096, 32, 128
NTOK = B * S                  # 8192
NCORES = 8
NT = NTOK // NCORES           # 1024 tokens per core
NJ = D_IN // HBS              # 32 hadamard blocks
QB = 16                       # quant block size
OG_N = D_OUT // 512           # 8 output column groups
H0 = 256                      # prologue half (tokens)

# quant snap constants (1.5*2^k magic so ulp is uniform on both sides of c)
C_HALF = 6291456.0            # 1.5*2^22, ulp 0.5
C_INT = 12582912.0            # 1.5*2^23, ulp 1.0
C_EVEN = 25165824.0           # 1.5*2^24, ulp 2.0
THF = 5.0625                  # 2.25^2
TH23 = 20.25                  # 4.5^2


# ---------------- custom DVE ops (e2m1 level snap, 2 passes, no t2) --------
def _register_snap_ops():
    from concourse.dve_spec import (
        Spec, Src0, Src1, C0, C1, C2, lower as dve_lower, sq, select, _has_src1,
    )
    from concourse.dve_ops import (
        DveOp, OPS, CUSTOM_DVE_SPECS, _SUB_OPCODE_FOR_NAME, _CUSTOM_DVE_ROW_BASE,
    )
    from concourse.dve_uop import DveOpSpec
    from concourse.dve_table_gen import dve_ver_for

    def _ref_a(in0, in1, c0, c1, c2):
        z = in0.astype(F32)
        r_int = (z + F32(c1)) - F32(c1)
        r_even = (z + F32(c2)) - F32(c2)
        return np.where(z * z < F32(c0), r_int, r_even).astype(F32)

    def _ref_b(in0, in1, c0, c1, c2):
        z = in0.astype(F32)
        r_half = (z + F32(c1)) - F32(c1)
        return np.where(z * z < F32(c0), r_half, in1.astype(F32)).astype(F32)

    def _mk(name, body, ref):
        if name in _SUB_OPCODE_FOR_NAME:
            return next(op for op in OPS if op.name == name)
        spec = Spec(body=body, reference=ref)
        row = _CUSTOM_DVE_ROW_BASE + len(OPS)
        assert row < 0x20
        ver = dve_ver_for("TRN2")
        uops = dve_lower(spec, ver=ver)
        sha = DveOpSpec(
            name=name, opcode=row, uops=uops, rd1_en=_has_src1(spec)
        ).sha(ver)
        op = DveOp(name, spec, subdim=False, uops_sha={ver: sha})
        OPS.append(op)
        CUSTOM_DVE_SPECS[name] = spec
        _SUB_OPCODE_FOR_NAME[name] = row
        return op

    z = Src0
    snap_a = _mk(
        "SNAP_A_ANT",
        select(sq(z) < C0, (z + C1) - C1, (z + C2) - C2),
        _ref_a,
    )
    snap_b = _mk(
        "SNAP_B_ANT",
        select(sq(z) < C0, (z + C1) - C1, Src1),
        _ref_b,
    )
    return snap_a, snap_b


SNAP_A, SNAP_B = _register_snap_ops()


# ---------------- device kernel ----------------
def _build_nc():
    nc = bacc.Bacc(
        "TRN2", target_bir_lowering=False, debug=False, num_devices=NCORES
    )
    dt = mybir.dt
    # x packed per (jg, chunk): [c, jg, seg] with seg = (dj, t) contiguous per
    # chunk (c0=128, c1=128, c2=768 tokens) so every DMA line is >=1KB
    xhi = nc.dram_tensor("xhi", [HBS, NJ // 4, 4 * NT], dt.bfloat16, kind="ExternalInput")
    xlo = nc.dram_tensor("xlo", [HBS, NJ // 4, 4 * NT], dt.bfloat16, kind="ExternalInput")
    Hs = nc.dram_tensor("Hs", [HBS, NJ, HBS], dt.bfloat16, kind="ExternalInput")
    laE = nc.dram_tensor("laE", [HBS, NJ, RANK], dt.bfloat16, kind="ExternalInput")
    w4 = nc.dram_tensor("w4", [HBS, OG_N, NJ, 512], dt.bfloat16, kind="ExternalInput")
    lbA = nc.dram_tensor("lbA", [RANK + 1, OG_N, 512], dt.bfloat16, kind="ExternalInput")
    y = nc.dram_tensor("y", [NT, D_OUT], dt.float32, kind="ExternalOutput")

    with tile.TileContext(nc) as tc:
        _emit(nc, tc, xhi, xlo, Hs, laE, w4, lbA, y)
    nc.compile()
    return nc


def _emit(nc, tc, xhi, xlo, Hs, laE, w4, lbA, y):
    from contextlib import ExitStack

    dt = mybir.dt
    Alu = mybir.AluOpType
    Act = mybir.ActivationFunctionType

    with ExitStack() as ctx:
        consts = ctx.enter_context(tc.tile_pool(name="consts", bufs=1))

        Hs_sb = consts.tile([HBS, NJ, HBS], dt.bfloat16)
        laE_sb = consts.tile([HBS, NJ, RANK], dt.bfloat16)
        nc.sync.dma_start(out=laE_sb[:], in_=laE[:])
        lbA_sb = consts.tile([RANK + 1, OG_N, 512], dt.bfloat16)
        nc.sync.dma_start(out=lbA_sb[:], in_=lbA[:])
        ident_bf = consts.tile([128, 128], dt.bfloat16)
        from concourse.masks import make_identity
        make_identity(nc, ident_bf[:])

        t1T = consts.tile([RANK + 1, NT], dt.bfloat16)
        nc.vector.memset(t1T[RANK : RANK + 1, :], 1.0)

        # xqT[c, j, t] : feature-major quantized activations (bf16)
        xqT = consts.tile([HBS, NJ, NT], dt.bfloat16)

        # working pools
        xh_pool = ctx.enter_context(tc.tile_pool(name="xh", bufs=2))
        xl_pool = ctx.enter_context(tc.tile_pool(name="xl", bufs=2))
        qsm = ctx.enter_context(tc.tile_pool(name="qsm", bufs=4))
        z_pool = ctx.enter_context(tc.tile_pool(name="z", bufs=3))
        r23_pool = ctx.enter_context(tc.tile_pool(name="r23", bufs=2))
        f_pool = ctx.enter_context(tc.tile_pool(name="f", bufs=3))
        xq_pool = ctx.enter_context(tc.tile_pool(name="xq", bufs=8))
        wbf_pool = ctx.enter_context(tc.tile_pool(name="wbf", bufs=2))
        out_pool = ctx.enter_context(tc.tile_pool(name="out", bufs=3))
        rot_ps = ctx.enter_context(tc.tile_pool(name="rotps", bufs=2, space="PSUM"))
        tr_ps = ctx.enter_context(tc.tile_pool(name="trps", bufs=2, space="PSUM"))
        t1_ps = ctx.enter_context(tc.tile_pool(name="t1ps", bufs=2, space="PSUM"))
        out_ps = ctx.enter_context(tc.tile_pool(name="outps", bufs=2, space="PSUM"))

        def emit_w_loads(h):
            tiles = []
            for og in range(OG_N):
                wt = wbf_pool.tile([HBS, NJ, 512], dt.bfloat16,
                                   name=f"w{h}_{og}", tag="wbf")
                nc.sync.dma_start(out=wt[:], in_=w4[:, og])
                tiles.append(wt)
            return tiles

        def emit_p1(h, t0, tlen, seg0, load_hs=False):
            hsl = slice(t0, t0 + tlen)
            ts_n = tlen // 128
            # LoRA1 accumulators in <=512-token chunks (one PSUM bank each)
            lchunks = []
            o = 0
            while o < tlen:
                w = min(512, tlen - o)
                acc = t1_ps.tile([RANK, w], dt.float32,
                                 name=f"t1acc{h}_{o}", tag="t1acc")
                lchunks.append((o, w, acc))
                o += w
            for jg in range(NJ // 4):
                seg = slice(seg0, seg0 + 4 * tlen)
                xh = xh_pool.tile([HBS, 4, tlen], dt.bfloat16,
                                  name=f"xh{h}_{jg}", tag="xh")
                nc.sync.dma_start(
                    out=xh[:],
                    in_=xhi[:, jg, seg].rearrange("c (d t) -> c d t", d=4),
                )
                xl = xl_pool.tile([HBS, 4, tlen], dt.bfloat16,
                                  name=f"xl{h}_{jg}", tag="xl")
                nc.sync.dma_start(
                    out=xl[:],
                    in_=xlo[:, jg, seg].rearrange("c (d t) -> c d t", d=4),
                )
                if load_hs:
                    nc.sync.dma_start(
                        out=Hs_sb[:, 4 * jg : 4 * jg + 4, :],
                        in_=Hs[:, 4 * jg : 4 * jg + 4, :],
                    )
                for dj in range(4):
                    j = 4 * jg + dj
                    for (o, w, acc) in lchunks:
                        nc.tensor.matmul(
                            acc[:], lhsT=laE_sb[:, j, :],
                            rhs=xh[:, dj, o : o + w],
                            start=(j == 0), stop=(j == NJ - 1),
                        )
                xq_tiles = []
                for ts in range(ts_n):
                    tsl = slice(ts * 128, (ts + 1) * 128)
                    bank = rot_ps.tile([128, 512], dt.float32,
                                       name=f"bank{h}_{jg}_{ts}", tag="bank")
                    for dj in range(4):
                        j = 4 * jg + dj
                        nc.tensor.matmul(
                            bank[:, dj * HBS : (dj + 1) * HBS],
                            lhsT=xh[:, dj, tsl], rhs=Hs_sb[:, j, :],
                            start=(dj == 0), stop=False,
                        )
                        nc.tensor.matmul(
                            bank[:, dj * HBS : (dj + 1) * HBS],
                            lhsT=xl[:, dj, tsl], rhs=Hs_sb[:, j, :],
                            start=False, stop=(dj == 3),
                        )
                    nb = 512 // QB
                    amax = qsm.tile([128, nb], dt.float32, name=f"am{h}{jg}{ts}", tag="amax")
                    nc.vector.tensor_reduce(
                        out=amax[:], in_=bank[:].rearrange("p (b s) -> p b s", s=QB),
                        axis=mybir.AxisListType.X, op=Alu.max,
                        apply_absolute_value=True,
                    )
                    ra = qsm.tile([128, nb], dt.float32, name=f"ra{h}{jg}{ts}", tag="ra")
                    nc.vector.reciprocal(out=ra[:], in_=amax[:])
                    rs6 = qsm.tile([128, nb], dt.float32, name=f"rs6{h}{jg}{ts}", tag="rs6")
                    nc.scalar.mul(out=rs6[:], in_=ra[:], mul=6.0)
                    sc = qsm.tile([128, nb], dt.float32, name=f"sc{h}{jg}{ts}", tag="sc")
                    nc.scalar.mul(out=sc[:], in_=amax[:], mul=1.0 / 6.0)
                    z = z_pool.tile([128, 512], dt.float32, name=f"z{h}{jg}{ts}", tag="z")
                    nc.vector.tensor_tensor(
                        out=z[:].rearrange("p (b s) -> p b s", s=QB),
                        in0=bank[:].rearrange("p (b s) -> p b s", s=QB),
                        in1=rs6[:].unsqueeze(2).broadcast_to([128, nb, QB]),
                        op=Alu.mult,
                    )
                    r23 = r23_pool.tile([128, 512], dt.float32, name=f"r23{h}{jg}{ts}", tag="r23")
                    nc.vector._custom_dve(
                        SNAP_A, out=r23[:], in0=z[:], s0=TH23, s1=C_INT, imm2=C_EVEN,
                    )
                    f = f_pool.tile([128, 512], dt.float32, name=f"f{h}{jg}{ts}", tag="f")
                    nc.vector._custom_dve(
                        SNAP_B, out=f[:], in0=z[:], in1=r23[:], s0=THF, s1=C_HALF,
                    )
                    xq_t = xq_pool.tile([128, 512], dt.bfloat16, name=f"xq{h}{jg}{ts}", tag="xq")
                    nc.gpsimd.tensor_tensor(
                        out=xq_t[:].rearrange("p (b s) -> p b s", s=QB),
                        in0=f[:].rearrange("p (b s) -> p b s", s=QB),
                        in1=sc[:].unsqueeze(2).broadcast_to([128, nb, QB]),
                        op=Alu.mult,
                    )
                    xq_tiles.append(xq_t)
                for dj in range(4):
                    j = 4 * jg + dj
                    pt = tr_ps.tile([128, tlen], dt.bfloat16, name=f"pt{h}_{j}", tag="pt")
                    for ts in range(ts_n):
                        nc.tensor.matmul(
                            pt[:, ts * 128 : (ts + 1) * 128],
                            lhsT=xq_tiles[ts][:, dj * HBS : (dj + 1) * HBS],
                            rhs=ident_bf[:], is_transpose=True,
                            start=(ts == 0), stop=(ts == ts_n - 1),
                        )
                    nc.scalar.copy(out=xqT[:, j, hsl], in_=pt[:])
            for (o, w, acc) in lchunks:
                nc.scalar.copy(out=t1T[:RANK, t0 + o : t0 + o + w], in_=acc[:])

        def emit_p2(h, t0, tlen, wtiles):
            ts_n = tlen // 128
            for og in range(OG_N):
                wt = wtiles[og]
                for th in range(ts_n):
                    tsl = slice(t0 + th * 128, t0 + (th + 1) * 128)
                    po = out_ps.tile([128, 512], dt.float32,
                                     name=f"po{h}{og}{th}", tag="po")
                    for k in range(NJ):
                        nc.tensor.matmul(
                            po[:], lhsT=xqT[:, k, tsl], rhs=wt[:, k, :],
                            start=(k == 0), stop=False,
                        )
                    nc.tensor.matmul(
                        po[:], lhsT=t1T[:, tsl], rhs=lbA_sb[:, og, :],
                        start=False, stop=True,
                    )
                    ot = out_pool.tile([128, 512], dt.float32,
                                       name=f"ot{h}{og}{th}", tag="ot")
                    nc.scalar.copy(out=ot[:], in_=po[:])
                    nc.sync.dma_start(
                        out=y[tsl, og * 512 : (og + 1) * 512], in_=ot[:]
                    )

        w0 = emit_w_loads(0)
        emit_p1(0, 0, 128, 0, load_hs=True)     # chunk c0
        emit_p1(1, 128, 128, 512)               # chunk c1
        emit_p1(2, H0, NT - H0, 1024)           # chunk c2 (= second half)
        w1 = emit_w_loads(1)
        emit_p2(0, 0, H0, w0)
        emit_p2(1, H0, NT - H0, w1)


_NC_CACHE = None


def _get_nc():
    global _NC_CACHE
    if _NC_CACHE is None:
        _NC_CACHE = _build_nc()
    return _NC_CACHE


# ---------------- host wrapper ----------------
def _prep_inputs(x, S_in, H_block, w_quantized, lora_a, lora_b, bias):
    import ml_dtypes
    BF16 = ml_dtypes.bfloat16
    x = np.asarray(x, dtype=F32)
    S_in = np.asarray(S_in, dtype=F32)
    H_block = np.asarray(H_block, dtype=F32)
    w_quantized = np.asarray(w_quantized, dtype=F32)
    lora_a = np.asarray(lora_a, dtype=F32)
    lora_b = np.asarray(lora_b, dtype=F32)
    bias = np.asarray(bias, dtype=F32)

    x_flat = x.reshape(NTOK, D_IN)
    x_hi = x_flat.astype(BF16)
    x_lo = (x_flat - x_hi.astype(F32)).astype(BF16)

    Ssq = S_in.reshape(NJ, HBS).T                        # [c, j]
    Hsign = np.sign(H_block).astype(F32)                 # +/-1 exact
    Hs = (Ssq[:, :, None] * Hsign[:, None, :]).astype(BF16)  # [c, j, c']

    la3 = lora_a.reshape(RANK, NJ, HBS)                  # [r, j, c']
    la_eff = np.einsum(
        "cd,rjd->cjr", H_block.astype(np.float64), la3.astype(np.float64)
    )
    laE = (Ssq[:, :, None] * la_eff.astype(F32)).astype(BF16)  # [c, j, r]

    rinv = np.float64(1.0) / np.sqrt(np.float64(HBS))
    # w4[c, og, j, o] = w[og*512+o, j*128+c] / sqrt(128)
    w4 = np.ascontiguousarray(
        (w_quantized.astype(np.float64) * rinv)
        .astype(F32)
        .reshape(OG_N, 512, NJ, HBS)
        .transpose(3, 0, 2, 1)
        .astype(BF16)
    )
    lbA = np.concatenate(
        [lora_b.T, bias.reshape(1, D_OUT)], axis=0
    ).reshape(RANK + 1, OG_N, 512).astype(BF16)
    lbA = np.ascontiguousarray(lbA)

    # chunk segmentation: (seg_offset, t0, tlen) — must match device emission
    CHUNKS = [(0, 0, 128), (512, 128, 128), (1024, 256, NT - 256)]

    def _pack(xv):
        # [NTOK, D] -> per-core [c, jg, seg] with seg=(chunk|dj|t) contiguous
        x5 = xv.reshape(NT, NJ, HBS).transpose(2, 1, 0)  # [c, j, t]
        x6 = np.empty((HBS, NJ // 4, 4 * NT), dtype=xv.dtype)
        for jg in range(NJ // 4):
            blk = x5[:, 4 * jg : 4 * jg + 4, :]           # [c, 4, NT]
            for (s0, t0, tl) in CHUNKS:
                x6[:, jg, s0 : s0 + 4 * tl] = blk[:, :, t0 : t0 + tl].reshape(HBS, 4 * tl)
        return np.ascontiguousarray(x6)

    per_core = []
    for c in range(NCORES):
        tsl = slice(c * NT, (c + 1) * NT)
        per_core.append(
            {"xhi": _pack(x_hi[tsl]), "xlo": _pack(x_lo[tsl]),
             "Hs": Hs, "laE": laE, "w4": w4, "lbA": lbA}
        )
    return per_core


def kernel(x, S_in, H_block, w_quantized, lora_a, lora_b, bias):
    in_maps = _prep_inputs(x, S_in, H_block, w_quantized, lora_a, lora_b, bias)
    nc = _get_nc()
    res = run_bass_kernel_spmd(nc, in_maps, core_ids=list(range(NCORES)))
    out = np.concatenate([res.results[c]["y"] for c in range(NCORES)], axis=0)
    return out.reshape(B, S, D_OUT).astype(F32)
